# revision 6
# baseline (speedup 1.0000x reference)
"""Trainium2 Bass kernel for ContrastiveLoss (N=16384, D=1024, 8 NeuronCores).

Strategy (data-parallel over anchors, transposed layout, fp8 HBM traffic):
  - Host shards anchor rows across 8 cores (2048 rows each), gathers pos/neg
    rows (gather commutes with row-wise normalization), casts to fp8e4,
    TRANSPOSES each block to [D, rows] = chunks of [128, 2048].
  - Device loads via SWDGE cast-DMAs (fp8 HBM -> fp16 SBUF): HBM reads are
    halved; the SBUF write side binds at the DMA-fabric rate. Loads are
    issued before the block barrier, pair-batched mid-stream, and split
    into column halves for the last chunk to shorten the tail.
  - Per chunk c (partitions = 128 dims, free = 2048 rows):
      DVE:  P_uv = U_c * V_c, P_uw = U_c * W_c   (fp16 tensor_tensor, 2x mode)
      ACT:  P_uu = Square(U_c)                   (fp16 activation)
      PE :  ones[128,32]^T @ P_s  -> psum[32s:32s+32, 512cg:512cg+512]
            accumulated over the 8 k-chunks (partition-axis reduction at
            ~N cycles/matmul -- far faster than DVE/ACT free-axis reduces).
  - Tail: the last chunk runs column-half granular; stats are extracted
    (ACT psum->sbuf copies) and stored (partition-strided 12 KB DMAs)
    per half as soon as their matmul groups retire.
  - Host epilogue (f64) reconstructs the reference math from raw-embedding
    dots:  a = u/max(|u|,eps),  ||a-b+eps||^2 ~= ahat2_a + ahat2_b
           - 2<u,v>/(den_a den_b) + D*eps^2, then the margin loss.
"""

import sys

for _p in ("/opt/trn_rl_repo", "/root/.axon_site/_ro/trn_rl_repo"):
    if _p not in sys.path:
        sys.path.append(_p)

import numpy as np
import ml_dtypes

N = 16384  # total rows
D = 1024  # embedding dim
NCORES = 8
RPC = N // NCORES  # rows per core = 2048
KC = D // 128  # k-chunks per core = 8
NPAIR = 3  # chunk pairs 0-5 loaded as [128, 2, 2048] DMAs
PSLOTS = 4  # product buffer slots per stat
NCG = RPC // 512  # 512-col matmul groups = 4
H = RPC // 2  # column half
EPS = 1e-6
MARGIN = 1.0

LAST_RESULT = None
_CACHE = {}


def _build_nc():
    import concourse.bass as bass
    import concourse.mybir as mybir

    f32 = mybir.dt.float32
    f16 = mybir.dt.float16
    fp8 = mybir.dt.float8e4
    nc = bass.Bass()
    ancp = nc.declare_dram_parameter("ancp", [NPAIR, 128, 2, RPC], fp8, isOutput=False)
    posp = nc.declare_dram_parameter("posp", [NPAIR, 128, 2, RPC], fp8, isOutput=False)
    negp = nc.declare_dram_parameter("negp", [NPAIR, 128, 2, RPC], fp8, isOutput=False)
    anc2 = nc.declare_dram_parameter("anc2", [2, 128, RPC], fp8, isOutput=False)
    pos2 = nc.declare_dram_parameter("pos2", [2, 128, RPC], fp8, isOutput=False)
    neg2 = nc.declare_dram_parameter("neg2", [2, 128, RPC], fp8, isOutput=False)
    one = nc.declare_dram_parameter("one", [128, 33], f16, isOutput=False)
    out = nc.declare_dram_parameter("out", [3, RPC], f32, isOutput=True)

    Sq = mybir.ActivationFunctionType.Square
    mult = mybir.AluOpType.mult

    from contextlib import ExitStack

    with ExitStack() as ctx:
        sb = lambda nm, shape, dt: ctx.enter_context(nc.sbuf_tensor(nm, shape, dt))
        ps_ = lambda nm, shape, dt: ctx.enter_context(nc.psum_tensor(nm, shape, dt))
        sem = lambda nm: ctx.enter_context(nc.semaphore(nm))

        U = sb("u", [128, KC, RPC], f16)
        V = sb("v", [128, KC, RPC], f16)
        W = sb("w", [128, KC, RPC], fp8)  # neg stays fp8; TT(U16, W8) runs 1x
        ONES = sb("ones", [128, 33], f16)  # cols 0-31 ones, col 32 zeros (Sq bias)
        PUV = sb("puv", [128, PSLOTS, RPC], f16)
        PUW = sb("puw", [128, PSLOTS, RPC], f16)
        PUU = sb("puu", [128, PSLOTS, RPC], f16)
        STATS = sb("stats", [96, RPC], f32)
        PS = ps_("ps", [96, RPC], f32)  # rows 0-31 uu, 32-63 uv, 64-95 uw

        SUP = [sem(f"sup{p}") for p in range(NPAIR)]
        SVP = [sem(f"svp{p}") for p in range(NPAIR)]
        SWP = [sem(f"swp{p}") for p in range(NPAIR)]
        SU6, SV6, SW6 = sem("su6"), sem("sv6"), sem("sw6")
        SU7 = [sem(f"su7{h}") for h in range(2)]
        SV7 = [sem(f"sv7{h}") for h in range(2)]
        SW7 = [sem(f"sw7{h}") for h in range(2)]
        s_one = sem("s_one")
        dve_uv = sem("dve_uv")  # +1 per TT uv, chunks 0-6
        dve_uw = sem("dve_uw")
        act_sq = sem("act_sq")  # +1 per Square, chunks 0-6
        d7_uv = sem("d7_uv")  # chunk-7 half TTs
        d7_uw = sem("d7_uw")
        a7_sq = sem("a7_sq")
        pe_uu = sem("pe_uu")  # +1 per finished (stat, chunk) group, chunks 0-6
        pe_uv = sem("pe_uv")
        pe_uw = sem("pe_uw")
        pe_h7 = sem("pe_h7")  # +1 per finished chunk-7 column-half
        ext_sem = sem("ext_sem")
        st_sem = sem("st_sem")

        # ---- loads issued before the block barrier ----
        nc.sync.dma_start(out=ONES[:], in_=one[:, :]).then_inc(s_one, 16)
        for p in range(NPAIR):
            nc.gpsimd.dma_start(out=U[:, 2 * p : 2 * p + 2, :], in_=ancp[p]).then_inc(
                SUP[p], 16
            )
            nc.gpsimd.dma_start(out=V[:, 2 * p : 2 * p + 2, :], in_=posp[p]).then_inc(
                SVP[p], 16
            )
            nc.sync.dma_start(out=W[:, 2 * p : 2 * p + 2, :], in_=negp[p]).then_inc(
                SWP[p], 16
            )
        nc.gpsimd.dma_start(out=U[:, 6, :], in_=anc2[0]).then_inc(SU6, 16)
        nc.gpsimd.dma_start(out=V[:, 6, :], in_=pos2[0]).then_inc(SV6, 16)
        nc.sync.dma_start(out=W[:, 6, :], in_=neg2[0]).then_inc(SW6, 16)
        # chunk 7: U and W first; V last so the tail chain is only TT_uv + MMs
        for h in range(2):
            cs = slice(H * h, H * h + H)
            nc.gpsimd.dma_start(out=U[:, 7, cs], in_=anc2[1][:, cs]).then_inc(
                SU7[h], 16
            )
            nc.sync.dma_start(out=W[:, 7, cs], in_=neg2[1][:, cs]).then_inc(
                SW7[h], 16
            )
        for h in range(2):
            cs = slice(H * h, H * h + H)
            nc.gpsimd.dma_start(out=V[:, 7, cs], in_=pos2[1][:, cs]).then_inc(
                SV7[h], 16
            )

        def u_wait(eng, c):
            if c < 6:
                eng.wait_ge(SUP[c // 2], 16)
            elif c == 6:
                eng.wait_ge(SU6, 16)

        def v_wait(eng, c):
            if c < 6:
                eng.wait_ge(SVP[c // 2], 16)
            elif c == 6:
                eng.wait_ge(SV6, 16)

        def w_wait(eng, c):
            if c < 6:
                eng.wait_ge(SWP[c // 2], 16)
            elif c == 6:
                eng.wait_ge(SW6, 16)

        block = ctx.enter_context(nc.Block())

        @block.sync
        def _(sync):
            for h in range(2):
                cs = slice(H * h, H * h + H)
                sync.wait_ge(ext_sem, h + 1)
                sync.dma_start(out=out[:, cs], in_=STATS[0:96:32, cs]).then_inc(
                    st_sem, 16
                )
            sync.wait_ge(st_sem, 32)

        @block.vector
        def _(vector):
            for c in range(7):
                sl = c % PSLOTS
                if c >= PSLOTS:
                    vector.wait_ge(pe_uv, c - PSLOTS + 1)
                u_wait(vector, c)
                v_wait(vector, c)
                nc.vector.tensor_tensor(
                    out=PUV[:, sl, :], in0=U[:, c, :], in1=V[:, c, :], op=mult
                ).then_inc(dve_uv, 1)
                if c >= PSLOTS:
                    vector.wait_ge(pe_uw, c - PSLOTS + 1)
                w_wait(vector, c)
                nc.vector.tensor_tensor(
                    out=PUW[:, sl, :], in0=U[:, c, :], in1=W[:, c, :], op=mult
                ).then_inc(dve_uw, 1)
            # chunk 7, column-half granular (slot 3)
            vector.wait_ge(pe_uv, 4)
            vector.wait_ge(pe_uw, 4)
            for h in range(2):
                cs = slice(H * h, H * h + H)
                vector.wait_ge(SU7[h], 16)
                vector.wait_ge(SW7[h], 16)
                nc.vector.tensor_tensor(
                    out=PUW[:, 3, cs], in0=U[:, 7, cs], in1=W[:, 7, cs], op=mult
                ).then_inc(d7_uw, 1)
            for h in range(2):
                cs = slice(H * h, H * h + H)
                vector.wait_ge(SV7[h], 16)
                nc.vector.tensor_tensor(
                    out=PUV[:, 3, cs], in0=U[:, 7, cs], in1=V[:, 7, cs], op=mult
                ).then_inc(d7_uv, 1)

        @block.scalar
        def _(scalar):
            for c in range(7):
                sl = c % PSLOTS
                if c >= PSLOTS:
                    scalar.wait_ge(pe_uu, c - PSLOTS + 1)
                u_wait(scalar, c)
                nc.scalar.activation(
                    out=PUU[:, sl, :], in_=U[:, c, :], func=Sq,
                    bias=ONES[:, 32:33],
                ).then_inc(act_sq, 1)
            scalar.wait_ge(pe_uu, 4)
            for h in range(2):
                cs = slice(H * h, H * h + H)
                scalar.wait_ge(SU7[h], 16)
                nc.scalar.activation(
                    out=PUU[:, 3, cs], in_=U[:, 7, cs], func=Sq,
                    bias=ONES[:, 32:33],
                ).then_inc(a7_sq, 1)
            # per-half extraction once that half's matmul groups retired
            for h in range(2):
                cs = slice(H * h, H * h + H)
                scalar.wait_ge(pe_h7, h + 1)
                nc.scalar.copy(out=STATS[0:96, cs], in_=PS[0:96, cs]).then_inc(
                    ext_sem, 1
                )

        @block.tensor
        def _(tensor):
            sems = (pe_uu, pe_uv, pe_uw)

            def mms(P, sl, s, cgs, c, inc=None):
                for i, cg in enumerate(cgs):
                    co = 512 * cg
                    mm = nc.tensor.matmul(
                        out=PS[32 * s : 32 * s + 32, co : co + 512],
                        lhsT=ONES[:, 0:32],
                        rhs=P[:, sl, co : co + 512],
                        start=(c == 0),
                        stop=(c == KC - 1),
                    )
                    if i == len(cgs) - 1 and inc is not None:
                        mm.then_inc(inc, 1)

            tensor.wait_ge(s_one, 16)
            for c in range(7):
                sl = c % PSLOTS
                tensor.wait_ge(act_sq, c + 1)
                mms(PUU, sl, 0, range(NCG), c, pe_uu)
                tensor.wait_ge(dve_uv, c + 1)
                mms(PUV, sl, 1, range(NCG), c, pe_uv)
                tensor.wait_ge(dve_uw, c + 1)
                mms(PUW, sl, 2, range(NCG), c, pe_uw)
            for h in range(2):
                cgs = (2 * h, 2 * h + 1)
                tensor.wait_ge(a7_sq, h + 1)
                mms(PUU, 3, 0, cgs, 7)
                tensor.wait_ge(d7_uw, h + 1)
                mms(PUW, 3, 2, cgs, 7)
                tensor.wait_ge(d7_uv, h + 1)
                mms(PUV, 3, 1, cgs, 7, pe_h7)

    return nc


def kernel(embeddings, labels, pos_idx, neg_idx):
    global LAST_RESULT
    from concourse.bass_utils import run_bass_kernel_spmd

    emb = np.asarray(embeddings, dtype=np.float32).astype(ml_dtypes.float8_e4m3)
    assert emb.shape == (N, D)
    pidx = np.asarray(pos_idx).astype(np.int64)
    nidx = np.asarray(neg_idx).astype(np.int64)
    ones = np.ones((128, 33), dtype=np.float16)
    ones[:, 32] = 0.0  # Square bias column

    def tchunks(rows):
        # [2048, 1024] -> chunks [8, 128, 2048]; pairs + last two chunks
        t = np.ascontiguousarray(rows.T).reshape(KC, 128, RPC)
        pairs = np.ascontiguousarray(
            t[:6].reshape(NPAIR, 2, 128, RPC).transpose(0, 2, 1, 3)
        )
        tail = np.ascontiguousarray(t[6:8])
        return pairs, tail

    in_maps = []
    for i in range(NCORES):
        sl = slice(i * RPC, (i + 1) * RPC)
        ap, a2 = tchunks(emb[sl])
        pp, p2 = tchunks(emb[pidx[sl]])
        np_, n2 = tchunks(emb[nidx[sl]])
        in_maps.append(
            {
                "ancp": ap, "anc2": a2,
                "posp": pp, "pos2": p2,
                "negp": np_, "neg2": n2,
                "one": ones,
            }
        )

    nc = _CACHE.get("nc")
    if nc is None:
        nc = _build_nc()
        _CACHE["nc"] = nc

    res = run_bass_kernel_spmd(nc, in_maps, list(range(NCORES)))
    LAST_RESULT = res

    def decode(k):
        return np.concatenate(
            [res.results[i]["out"][k] for i in range(NCORES)]
        ).astype(np.float64)

    nu2 = decode(0)
    P = decode(1)
    Q = decode(2)

    norm = np.sqrt(nu2)
    den = np.maximum(norm, EPS)  # F.normalize clamp
    ahat2 = nu2 / (den * den)  # ||a_hat||^2 (==1 unless degenerate)

    def dist(idx, dot):
        S = ahat2 + ahat2[idx] - 2.0 * dot / (den * den[idx]) + D * EPS * EPS
        return np.sqrt(np.maximum(S, 0.0)) + EPS

    d_pos = dist(pidx, P)
    d_neg = dist(nidx, Q)
    pos_loss = d_pos * d_pos
    neg_loss = np.maximum(MARGIN - d_neg, EPS) ** 2
    total = pos_loss.sum() + neg_loss.sum()
    return np.array(total / (2.0 * N), dtype=np.float32)


# revision 7
# speedup vs baseline: 1.1459x; 1.1459x over previous
"""Trainium2 Bass kernel for ContrastiveLoss (N=16384, D=1024, 8 NeuronCores).

Strategy (data-parallel over anchors, transposed layout, fp8 HBM traffic):
  - Host shards anchor rows across 8 cores (2048 rows each), gathers pos/neg
    rows (gather commutes with row-wise normalization), casts to fp8e4,
    TRANSPOSES each block to [D, rows] = chunks of [128, 2048].
  - Device loads via SWDGE cast-DMAs (fp8 HBM -> fp16 SBUF): HBM reads are
    halved; the SBUF write side binds at the DMA-fabric rate. Loads are
    issued before the block barrier, pair-batched mid-stream, and split
    into column halves for the last chunk to shorten the tail.
  - Per chunk c (partitions = 128 dims, free = 2048 rows):
      DVE:  P_uv = U_c * V_c, P_uw = U_c * W_c   (fp16 tensor_tensor, 2x mode)
      ACT:  P_uu = Square(U_c)                   (fp16 activation)
      PE :  ones[128,32]^T @ P_s  -> psum[32s:32s+32, 512cg:512cg+512]
            accumulated over the 8 k-chunks (partition-axis reduction at
            ~N cycles/matmul -- far faster than DVE/ACT free-axis reduces).
  - Tail: the last chunk runs column-half granular; stats are extracted
    (ACT psum->sbuf copies) and stored (partition-strided 12 KB DMAs)
    per half as soon as their matmul groups retire.
  - Host epilogue (f64) reconstructs the reference math from raw-embedding
    dots:  a = u/max(|u|,eps),  ||a-b+eps||^2 ~= ahat2_a + ahat2_b
           - 2<u,v>/(den_a den_b) + D*eps^2, then the margin loss.
"""

import sys

for _p in ("/opt/trn_rl_repo", "/root/.axon_site/_ro/trn_rl_repo"):
    if _p not in sys.path:
        sys.path.append(_p)

import numpy as np
import ml_dtypes

N = 16384  # total rows
D = 1024  # embedding dim
NCORES = 8
RPC = N // NCORES  # rows per core = 2048
KC = D // 128  # k-chunks per core = 8
NPAIR = 3  # chunk pairs 0-5 loaded as [128, 2, 2048] DMAs
PSLOTS = 4  # product buffer slots per stat
NCG = RPC // 512  # 512-col matmul groups = 4
H = RPC // 2  # column half
EPS = 1e-6
MARGIN = 1.0

LAST_RESULT = None
_CACHE = {}


def _build_nc():
    import concourse.bass as bass
    import concourse.mybir as mybir

    f32 = mybir.dt.float32
    f16 = mybir.dt.float16
    fp8 = mybir.dt.float8e4
    nc = bass.Bass()
    ancp = nc.declare_dram_parameter("ancp", [NPAIR, 128, 2, RPC], fp8, isOutput=False)
    posp = nc.declare_dram_parameter("posp", [NPAIR, 128, 2, RPC], fp8, isOutput=False)
    negp = nc.declare_dram_parameter("negp", [NPAIR, 128, 2, RPC], fp8, isOutput=False)
    anc2 = nc.declare_dram_parameter("anc2", [2, 128, RPC], fp8, isOutput=False)
    pos2 = nc.declare_dram_parameter("pos2", [2, 128, RPC], fp8, isOutput=False)
    neg2 = nc.declare_dram_parameter("neg2", [2, 128, RPC], fp8, isOutput=False)
    one = nc.declare_dram_parameter("one", [128, 33], f16, isOutput=False)
    out = nc.declare_dram_parameter("out", [3, RPC], f32, isOutput=True)

    Sq = mybir.ActivationFunctionType.Square
    mult = mybir.AluOpType.mult

    from contextlib import ExitStack

    with ExitStack() as ctx:
        sb = lambda nm, shape, dt: ctx.enter_context(nc.sbuf_tensor(nm, shape, dt))
        ps_ = lambda nm, shape, dt: ctx.enter_context(nc.psum_tensor(nm, shape, dt))
        sem = lambda nm: ctx.enter_context(nc.semaphore(nm))

        U = sb("u", [128, KC, RPC], f16)
        V = sb("v", [128, KC, RPC], f16)
        W = sb("w", [128, KC, RPC], f16)
        ONES = sb("ones", [128, 33], f16)  # cols 0-31 ones, col 32 zeros (Sq bias)
        PUV = sb("puv", [128, PSLOTS, RPC], f16)
        PUW = sb("puw", [128, PSLOTS, RPC], f16)
        PUU = sb("puu", [128, PSLOTS, RPC], f16)
        STATS = sb("stats", [96, RPC], f32)
        PS = ps_("ps", [96, RPC], f32)  # rows 0-31 uu, 32-63 uv, 64-95 uw

        SUP = [sem(f"sup{p}") for p in range(NPAIR)]
        SVP = [sem(f"svp{p}") for p in range(NPAIR)]
        SWP = [sem(f"swp{p}") for p in range(NPAIR)]
        SU6, SV6, SW6 = sem("su6"), sem("sv6"), sem("sw6")
        SU7 = [sem(f"su7{h}") for h in range(2)]
        SV7 = [sem(f"sv7{h}") for h in range(2)]
        SW7 = [sem(f"sw7{h}") for h in range(2)]
        s_one = sem("s_one")
        dve_uv = sem("dve_uv")  # +1 per TT uv, chunks 0-6
        dve_uw = sem("dve_uw")
        act_sq = sem("act_sq")  # +1 per Square, chunks 0-6
        d7_uv = sem("d7_uv")  # chunk-7 half TTs
        d7_uw = sem("d7_uw")
        a7_sq = sem("a7_sq")
        pe_uu = sem("pe_uu")  # +1 per finished (stat, chunk) group, chunks 0-6
        pe_uv = sem("pe_uv")
        pe_uw = sem("pe_uw")
        pe_h7 = sem("pe_h7")  # +1 per finished chunk-7 column-half
        ext_sem = sem("ext_sem")
        st_sem = sem("st_sem")

        # ---- loads issued before the block barrier ----
        nc.sync.dma_start(out=ONES[:], in_=one[:, :]).then_inc(s_one, 16)
        for p in range(NPAIR):
            nc.gpsimd.dma_start(out=U[:, 2 * p : 2 * p + 2, :], in_=ancp[p]).then_inc(
                SUP[p], 16
            )
            nc.gpsimd.dma_start(out=V[:, 2 * p : 2 * p + 2, :], in_=posp[p]).then_inc(
                SVP[p], 16
            )
            nc.gpsimd.dma_start(out=W[:, 2 * p : 2 * p + 2, :], in_=negp[p]).then_inc(
                SWP[p], 16
            )
        nc.gpsimd.dma_start(out=U[:, 6, :], in_=anc2[0]).then_inc(SU6, 16)
        nc.gpsimd.dma_start(out=V[:, 6, :], in_=pos2[0]).then_inc(SV6, 16)
        nc.gpsimd.dma_start(out=W[:, 6, :], in_=neg2[0]).then_inc(SW6, 16)
        # chunk 7: U and W first; V last so the tail chain is only TT_uv + MMs
        for h in range(2):
            cs = slice(H * h, H * h + H)
            nc.gpsimd.dma_start(out=U[:, 7, cs], in_=anc2[1][:, cs]).then_inc(
                SU7[h], 16
            )
            nc.gpsimd.dma_start(out=W[:, 7, cs], in_=neg2[1][:, cs]).then_inc(
                SW7[h], 16
            )
        for h in range(2):
            cs = slice(H * h, H * h + H)
            nc.gpsimd.dma_start(out=V[:, 7, cs], in_=pos2[1][:, cs]).then_inc(
                SV7[h], 16
            )

        def u_wait(eng, c):
            if c < 6:
                eng.wait_ge(SUP[c // 2], 16)
            elif c == 6:
                eng.wait_ge(SU6, 16)

        def v_wait(eng, c):
            if c < 6:
                eng.wait_ge(SVP[c // 2], 16)
            elif c == 6:
                eng.wait_ge(SV6, 16)

        def w_wait(eng, c):
            if c < 6:
                eng.wait_ge(SWP[c // 2], 16)
            elif c == 6:
                eng.wait_ge(SW6, 16)

        block = ctx.enter_context(nc.Block())

        @block.sync
        def _(sync):
            for h in range(2):
                cs = slice(H * h, H * h + H)
                sync.wait_ge(ext_sem, h + 1)
                sync.dma_start(out=out[:, cs], in_=STATS[0:96:32, cs]).then_inc(
                    st_sem, 16
                )
            sync.wait_ge(st_sem, 32)

        @block.vector
        def _(vector):
            for c in range(7):
                sl = c % PSLOTS
                if c >= PSLOTS:
                    vector.wait_ge(pe_uv, c - PSLOTS + 1)
                u_wait(vector, c)
                v_wait(vector, c)
                nc.vector.tensor_tensor(
                    out=PUV[:, sl, :], in0=U[:, c, :], in1=V[:, c, :], op=mult
                ).then_inc(dve_uv, 1)
                if c >= PSLOTS:
                    vector.wait_ge(pe_uw, c - PSLOTS + 1)
                w_wait(vector, c)
                nc.vector.tensor_tensor(
                    out=PUW[:, sl, :], in0=U[:, c, :], in1=W[:, c, :], op=mult
                ).then_inc(dve_uw, 1)
            # chunk 7, column-half granular (slot 3)
            vector.wait_ge(pe_uv, 4)
            vector.wait_ge(pe_uw, 4)
            for h in range(2):
                cs = slice(H * h, H * h + H)
                vector.wait_ge(SU7[h], 16)
                vector.wait_ge(SW7[h], 16)
                nc.vector.tensor_tensor(
                    out=PUW[:, 3, cs], in0=U[:, 7, cs], in1=W[:, 7, cs], op=mult
                ).then_inc(d7_uw, 1)
            for h in range(2):
                cs = slice(H * h, H * h + H)
                vector.wait_ge(SV7[h], 16)
                nc.vector.tensor_tensor(
                    out=PUV[:, 3, cs], in0=U[:, 7, cs], in1=V[:, 7, cs], op=mult
                ).then_inc(d7_uv, 1)

        @block.scalar
        def _(scalar):
            for c in range(7):
                sl = c % PSLOTS
                if c >= PSLOTS:
                    scalar.wait_ge(pe_uu, c - PSLOTS + 1)
                u_wait(scalar, c)
                nc.scalar.activation(
                    out=PUU[:, sl, :], in_=U[:, c, :], func=Sq,
                    bias=ONES[:, 32:33],
                ).then_inc(act_sq, 1)
            scalar.wait_ge(pe_uu, 4)
            for h in range(2):
                cs = slice(H * h, H * h + H)
                scalar.wait_ge(SU7[h], 16)
                nc.scalar.activation(
                    out=PUU[:, 3, cs], in_=U[:, 7, cs], func=Sq,
                    bias=ONES[:, 32:33],
                ).then_inc(a7_sq, 1)
            # per-half extraction once that half's matmul groups retired
            for h in range(2):
                cs = slice(H * h, H * h + H)
                scalar.wait_ge(pe_h7, h + 1)
                nc.scalar.copy(out=STATS[0:96, cs], in_=PS[0:96, cs]).then_inc(
                    ext_sem, 1
                )

        @block.tensor
        def _(tensor):
            sems = (pe_uu, pe_uv, pe_uw)

            def mms(P, sl, s, cgs, c, inc=None):
                for i, cg in enumerate(cgs):
                    co = 512 * cg
                    mm = nc.tensor.matmul(
                        out=PS[32 * s : 32 * s + 32, co : co + 512],
                        lhsT=ONES[:, 0:32],
                        rhs=P[:, sl, co : co + 512],
                        start=(c == 0),
                        stop=(c == KC - 1),
                    )
                    if i == len(cgs) - 1 and inc is not None:
                        mm.then_inc(inc, 1)

            tensor.wait_ge(s_one, 16)
            for c in range(7):
                sl = c % PSLOTS
                tensor.wait_ge(act_sq, c + 1)
                mms(PUU, sl, 0, range(NCG), c, pe_uu)
                tensor.wait_ge(dve_uv, c + 1)
                mms(PUV, sl, 1, range(NCG), c, pe_uv)
                tensor.wait_ge(dve_uw, c + 1)
                mms(PUW, sl, 2, range(NCG), c, pe_uw)
            for h in range(2):
                cgs = (2 * h, 2 * h + 1)
                tensor.wait_ge(a7_sq, h + 1)
                mms(PUU, 3, 0, cgs, 7)
                tensor.wait_ge(d7_uw, h + 1)
                mms(PUW, 3, 2, cgs, 7)
                tensor.wait_ge(d7_uv, h + 1)
                mms(PUV, 3, 1, cgs, 7, pe_h7)

    return nc


def kernel(embeddings, labels, pos_idx, neg_idx):
    global LAST_RESULT
    from concourse.bass_utils import run_bass_kernel_spmd

    emb = np.asarray(embeddings, dtype=np.float32).astype(ml_dtypes.float8_e4m3)
    assert emb.shape == (N, D)
    pidx = np.asarray(pos_idx).astype(np.int64)
    nidx = np.asarray(neg_idx).astype(np.int64)
    ones = np.ones((128, 33), dtype=np.float16)
    ones[:, 32] = 0.0  # Square bias column

    def tchunks(rows):
        # [2048, 1024] -> chunks [8, 128, 2048]; pairs + last two chunks
        t = np.ascontiguousarray(rows.T).reshape(KC, 128, RPC)
        pairs = np.ascontiguousarray(
            t[:6].reshape(NPAIR, 2, 128, RPC).transpose(0, 2, 1, 3)
        )
        tail = np.ascontiguousarray(t[6:8])
        return pairs, tail

    in_maps = []
    for i in range(NCORES):
        sl = slice(i * RPC, (i + 1) * RPC)
        ap, a2 = tchunks(emb[sl])
        pp, p2 = tchunks(emb[pidx[sl]])
        np_, n2 = tchunks(emb[nidx[sl]])
        in_maps.append(
            {
                "ancp": ap, "anc2": a2,
                "posp": pp, "pos2": p2,
                "negp": np_, "neg2": n2,
                "one": ones,
            }
        )

    nc = _CACHE.get("nc")
    if nc is None:
        nc = _build_nc()
        _CACHE["nc"] = nc

    res = run_bass_kernel_spmd(nc, in_maps, list(range(NCORES)))
    LAST_RESULT = res

    def decode(k):
        return np.concatenate(
            [res.results[i]["out"][k] for i in range(NCORES)]
        ).astype(np.float64)

    nu2 = decode(0)
    P = decode(1)
    Q = decode(2)

    norm = np.sqrt(nu2)
    den = np.maximum(norm, EPS)  # F.normalize clamp
    ahat2 = nu2 / (den * den)  # ||a_hat||^2 (==1 unless degenerate)

    def dist(idx, dot):
        S = ahat2 + ahat2[idx] - 2.0 * dot / (den * den[idx]) + D * EPS * EPS
        return np.sqrt(np.maximum(S, 0.0)) + EPS

    d_pos = dist(pidx, P)
    d_neg = dist(nidx, Q)
    pos_loss = d_pos * d_pos
    neg_loss = np.maximum(MARGIN - d_neg, EPS) ** 2
    total = pos_loss.sum() + neg_loss.sum()
    return np.array(total / (2.0 * N), dtype=np.float32)
